# revision 1
# baseline (speedup 1.0000x reference)
"""GSA block kernel for 8 NeuronCores.

Sharding: 16 (batch, head-pair) units -> core c handles batch b=c//2 and
heads {2*(c%2), 2*(c%2)+1}. Recurrence is computed chunkwise (chunk=64):
within-chunk contributions via causal matmuls, cross-chunk via a scan over
32 chunk states. Final o @ Wo is done on host after gathering heads.
"""
import numpy as np
from functools import partial

B, T, D = 4, 2048, 1024
H, K, V, M = 4, 256, 256, 64
GATE_NORM = 8.0
NORM_EPS = 1e-5
SCALE = K ** -0.5
C = 64           # chunk length
NCH = T // C     # 32 chunks
HP = 2           # heads per core


def _chunk_math(jnp, jax, hk, hv, qt, kt, vt, st, ft, mask):
    # qt,kt: [HP,C,K]; vt: [HP,C,V]; st,ft: [HP,C,M]; hk: [HP,K,M]; hv: [HP,M,V]
    Ai = jnp.exp(jnp.cumsum(ft, axis=1))            # inclusive cumprod of gates
    atot = Ai[:, -1, :]                             # [HP,M]
    stil = st / Ai                                  # s_i / A_i
    QK = jnp.einsum('hck,hdk->hcd', qt, kt) * SCALE
    QKm = jnp.where(mask, QK, 0.0)
    logits = Ai * (jnp.einsum('hck,hkm->hcm', qt * SCALE, hk)
                   + jnp.einsum('hcd,hdm->hcm', QKm, stil))
    p = jax.nn.softmax(logits, axis=-1)
    pt = p * Ai
    PS = jnp.einsum('hcm,hdm->hcd', pt, stil)
    PSm = jnp.where(mask, PS, 0.0)
    o = (jnp.einsum('hcm,hmv->hcv', pt, hv)
         + jnp.einsum('hcd,hdv->hcv', PSm, vt))
    sa = stil * atot[:, None, :]
    hk2 = hk * atot[:, None, :] + jnp.einsum('hck,hcm->hkm', kt, sa)
    hv2 = hv * atot[:, :, None] + jnp.einsum('hcm,hcv->hmv', sa, vt)
    return hk2, hv2, o


def _make_core_fn(jax, jnp):
    def core_fn(x, Wq, Wk, Wv, Wf, gw):
        # x: [T,D]; Wq/Wk/Wv: [D, HP*K]; Wf: [D, HP*M]; gw: [V]
        sig = jax.nn.sigmoid
        q = (lambda y: y * sig(y))(x @ Wq).reshape(T, HP, K)
        k = (lambda y: y * sig(y))(x @ Wk).reshape(T, HP, K)
        v = (x @ Wv).reshape(T, HP, V)
        f = jax.nn.log_sigmoid(x @ Wf).reshape(T, HP, M) / GATE_NORM
        s = 1.0 - jnp.exp(f)

        def to_chunks(a):
            return a.reshape(NCH, C, HP, -1).transpose(0, 2, 1, 3)

        qc, kc, vc, sc, fc = map(to_chunks, (q, k, v, s, f))
        mask = jnp.tril(jnp.ones((C, C), bool))

        def step(carry, inp):
            hk, hv = carry
            hk2, hv2, o = _chunk_math(jnp, jax, hk, hv, *inp, mask)
            return (hk2, hv2), o

        init = (jnp.zeros((HP, K, M), jnp.float32),
                jnp.zeros((HP, M, V), jnp.float32))
        _, o = jax.lax.scan(step, init, (qc, kc, vc, sc, fc))
        o = o.transpose(0, 2, 1, 3).reshape(T, HP, V)
        o = o * jax.lax.rsqrt(jnp.mean(o * o, axis=-1, keepdims=True) + NORM_EPS)
        return o * gw
    return core_fn


def _shard_inputs(x, Wq, Wk, Wv, Wf, g_norm_w):
    xs = np.stack([x[c // 2] for c in range(8)])
    def wsh(W, span):
        return np.stack([W[:, (c % 2) * HP * span:((c % 2) + 1) * HP * span]
                         for c in range(8)])
    return (xs, wsh(Wq, K), wsh(Wk, K), wsh(Wv, K), wsh(Wf, M),
            np.broadcast_to(g_norm_w, (8, V)).copy())


def _run_device(x, Wq, Wk, Wv, Wf, g_norm_w):
    import jax
    import jax.numpy as jnp
    core_fn = _make_core_fn(jax, jnp)
    pm = jax.pmap(core_fn)
    res = np.asarray(pm(*_shard_inputs(x, Wq, Wk, Wv, Wf, g_norm_w)))
    return res  # [8, T, HP, V]


def _run_numpy(x, Wq, Wk, Wv, Wf, g_norm_w):
    # Pure-numpy fallback mirroring the same chunkwise math.
    class _J:  # minimal jax shim
        class nn:
            @staticmethod
            def sigmoid(z): return 1.0 / (1.0 + np.exp(-z))
            @staticmethod
            def log_sigmoid(z): return -np.logaddexp(0.0, -z)
            @staticmethod
            def softmax(z, axis=-1):
                z = z - z.max(axis=axis, keepdims=True)
                e = np.exp(z)
                return e / e.sum(axis=axis, keepdims=True)
    jnp_like = np
    res = np.zeros((8, T, HP, V), np.float32)
    shards = _shard_inputs(x, Wq, Wk, Wv, Wf, g_norm_w)
    mask = np.tril(np.ones((C, C), bool))
    for c in range(8):
        xc, wq, wk, wv, wf, gw = (a[c] for a in shards)
        sig = _J.nn.sigmoid
        q = (lambda y: y * sig(y))(xc @ wq).reshape(T, HP, K)
        k = (lambda y: y * sig(y))(xc @ wk).reshape(T, HP, K)
        v = (xc @ wv).reshape(T, HP, V)
        f = _J.nn.log_sigmoid(xc @ wf).reshape(T, HP, M) / GATE_NORM
        s = 1.0 - np.exp(f)
        qc, kc, vc, sc, fc = (a.reshape(NCH, C, HP, -1).transpose(0, 2, 1, 3)
                              for a in (q, k, v, s, f))
        hk = np.zeros((HP, K, M), np.float32)
        hv = np.zeros((HP, M, V), np.float32)
        out = np.zeros((NCH, HP, C, V), np.float32)
        for i in range(NCH):
            hk, hv, out[i] = _chunk_math(jnp_like, _J, hk, hv,
                                         qc[i], kc[i], vc[i], sc[i], fc[i], mask)
        o = out.transpose(0, 2, 1, 3).reshape(T, HP, V)
        o = o / np.sqrt((o * o).mean(axis=-1, keepdims=True) + NORM_EPS)
        res[c] = o * gw
    return res


def kernel(x, Wq, Wk, Wv, Wf, g_norm_w, Wo):
    x = np.asarray(x, np.float32)
    args = (x, np.asarray(Wq, np.float32), np.asarray(Wk, np.float32),
            np.asarray(Wv, np.float32), np.asarray(Wf, np.float32),
            np.asarray(g_norm_w, np.float32))
    # XLA->neuronxcc in this environment cannot compile this graph (internal
    # error in lower_act on log_sigmoid), so the device path is not attempted.
    res = _run_numpy(*args)
    # res: [8, T, HP, V] -> o_full: [B, T, H, V]
    o_full = np.empty((B, T, H, V), np.float32)
    for c in range(8):
        b, hp = c // 2, c % 2
        o_full[b, :, 2 * hp:2 * hp + 2, :] = res[c]
    return o_full.reshape(B, T, H * V) @ np.asarray(Wo, np.float32)



# revision 2
# speedup vs baseline: 2.4973x; 2.4973x over previous
"""GSA block kernel for 8 trn2 NeuronCores (Bass/Tile via PJRT).

Sharding: core c -> batch b=c//2, head-pair hp=c%2 (heads {2hp, 2hp+1}).
Wire fp16: per-core "io" (ExternalOutput donated with x half-batch rows as
its initial value; final out D-column half written back) + "wblob" (1/4
slice of the per-hp weight blob). On-device: pair AllGather rebuilds x[b];
quad AllGather ({0,2,4,6}/{1,3,5,7}) rebuilds the hp weight blob; chunkwise
(C=128) recurrence in fp32; pair AllGather of o_n^T; each core computes its
512 output columns (Wo half rides in the hp blob, so the program is
core-independent). Falls back to a pure-numpy path if the device path is
unavailable.
"""
import contextlib
import numpy as np

B, T, D = 4, 2048, 1024
H, K, V, M = 4, 256, 256, 64
HP = 2
FQ = HP * K              # 512
FM = HP * M              # 128
CC = 128                 # device chunk length
NCH = T // CC            # 16
SCALE = K ** -0.5
EPS = 1e-5
TH = T // 2              # 1024

OQ, OKk, OV, OF, OO = 0, 512, 1024, 1536, 1664
WCOLS = 2176
WROWS = 256

PAIRS = [[0, 1], [2, 3], [4, 5], [6, 7]]
QUADS = [[0, 2, 4, 6], [1, 3, 5, 7]]


def _build():
    import concourse.mybir as mybir
    import concourse.tile as tile
    from concourse import bacc
    from concourse.masks import make_identity, make_upper_triangular

    F16 = mybir.dt.float16
    F32 = mybir.dt.float32
    AF = mybir.ActivationFunctionType
    ALU = mybir.AluOpType
    AX = mybir.AxisListType

    nc = bacc.Bacc("TRN2", target_bir_lowering=False, debug=False, num_devices=8)
    wblob = nc.dram_tensor("wblob", [WROWS, WCOLS], F16, kind="ExternalInput").ap()
    io = nc.dram_tensor("io", [TH, D], F16, kind="ExternalOutput").ap()

    with tile.TileContext(nc) as tc, contextlib.ExitStack() as ctx:
        dram = ctx.enter_context(tc.tile_pool(name="dram", bufs=1, space="DRAM"))
        consts = ctx.enter_context(tc.tile_pool(name="consts", bufs=1))

        xh = dram.tile([TH, D], F16)
        nc.sync.dma_start(out=xh[:], in_=io[:])
        xb = dram.tile([T, D], F16)
        nc.gpsimd.collective_compute(
            "AllGather", ALU.bypass, replica_groups=PAIRS,
            ins=[xh.opt()], outs=[xb.opt()])
        wh = dram.tile([WROWS, WCOLS], F16)
        nc.sync.dma_start(out=wh[:], in_=wblob[:])
        wf = dram.tile([4 * WROWS, WCOLS], F16)
        nc.gpsimd.collective_compute(
            "AllGather", ALU.bypass, replica_groups=QUADS,
            ins=[wh.opt()], outs=[wf.opt()])

        U = consts.tile([128, 128], F32)
        make_upper_triangular(nc, U[:], val=1.0, diag=True)
        ident = consts.tile([128, 128], F32)
        make_identity(nc, ident[:])
        ones = consts.tile([128, 128], F32)
        nc.vector.memset(ones[:], 1.0)
        epsc = consts.tile([128, 1], F32)
        nc.vector.memset(epsc[:], EPS)

        xT_pool = ctx.enter_context(tc.tile_pool(name="xTp", bufs=1))
        xT = xT_pool.tile([128, 8, T], F16)
        for d8 in range(8):
            nc.sync.dma_start(out=xT[:, d8], in_=xb[:, d8 * 128:(d8 + 1) * 128],
                              transpose=True)

        res_pool = ctx.enter_context(tc.tile_pool(name="resp", bufs=1))
        qT = res_pool.tile([128, 4, T], F32)
        kT = res_pool.tile([128, 4, T], F32)
        fT = res_pool.tile([128, T], F32)
        kc = res_pool.tile([128, NCH, FQ], F16)
        vc = res_pool.tile([128, NCH, FQ], F16)
        fc = res_pool.tile([128, NCH, FM], F32)
        on16 = res_pool.tile([128, NCH, FQ], F16)

        with tc.tile_pool(name="pjw", bufs=10) as pjw, \
             tc.tile_pool(name="pjwa", bufs=1) as pjwa, \
             tc.tile_pool(name="pjps", bufs=4, space="PSUM") as pjps:

            def projB(colbase, nf, evict):
                for ft in range(nf):
                    wts = []
                    for d8 in range(8):
                        w_ = pjw.tile([128, 128], F16, tag="pjw")
                        nc.sync.dma_start(
                            out=w_[:],
                            in_=wf[d8 * 128:(d8 + 1) * 128,
                                   colbase + ft * 128:colbase + (ft + 1) * 128])
                        wts.append(w_)
                    for tk in range(4):
                        ps = pjps.tile([128, 512], F32, tag="pjps")
                        for d8 in range(8):
                            nc.tensor.matmul(ps[:], wts[d8][:],
                                             xT[:, d8, tk * 512:(tk + 1) * 512],
                                             start=(d8 == 0), stop=(d8 == 7))
                        evict(ps, ft, tk)

            def ev_q(ps, ft, tk):
                sl = qT[:, ft, tk * 512:(tk + 1) * 512]
                nc.scalar.activation(sl, ps[:], AF.Silu)
                nc.vector.tensor_scalar_mul(sl, sl, SCALE)

            def ev_k(ps, ft, tk):
                nc.scalar.activation(kT[:, ft, tk * 512:(tk + 1) * 512], ps[:],
                                     AF.Silu)

            def ev_fT(ps, ft, tk):
                sl = fT[:, tk * 512:(tk + 1) * 512]
                nc.scalar.activation(sl, ps[:], AF.Sigmoid)
                nc.scalar.activation(sl, sl, AF.Ln)
                nc.vector.tensor_scalar_mul(sl, sl, 0.125)

            projB(OQ, 4, ev_q)
            projB(OKk, 4, ev_k)
            projB(OF, 1, ev_fT)

            wkA = pjwa.tile([128, 8, FQ], F16)
            wvA = pjwa.tile([128, 8, FQ], F16)
            wfA = pjwa.tile([128, 8, FM], F16)
            for d8 in range(8):
                r = slice(d8 * 128, (d8 + 1) * 128)
                nc.sync.dma_start(out=wkA[:, d8], in_=wf[r, OKk:OKk + FQ])
                nc.sync.dma_start(out=wvA[:, d8], in_=wf[r, OV:OV + FQ])
                nc.sync.dma_start(out=wfA[:, d8], in_=wf[r, OF:OF + FM])
            for tt in range(NCH):
                tsl = slice(tt * 128, (tt + 1) * 128)
                psk = pjps.tile([128, FQ], F32, tag="pjps")
                for d8 in range(8):
                    nc.tensor.matmul(psk[:], xT[:, d8, tsl], wkA[:, d8],
                                     start=(d8 == 0), stop=(d8 == 7))
                nc.scalar.activation(kc[:, tt], psk[:], AF.Silu)
                psv = pjps.tile([128, FQ], F32, tag="pjps")
                for d8 in range(8):
                    nc.tensor.matmul(psv[:], xT[:, d8, tsl], wvA[:, d8],
                                     start=(d8 == 0), stop=(d8 == 7))
                nc.scalar.activation(vc[:, tt], psv[:], AF.Copy)
                psf = pjps.tile([128, FM], F32, tag="pjps")
                for d8 in range(8):
                    nc.tensor.matmul(psf[:], xT[:, d8, tsl], wfA[:, d8],
                                     start=(d8 == 0), stop=(d8 == 7))
                nc.scalar.activation(fc[:, tt], psf[:], AF.Sigmoid)
                nc.scalar.activation(fc[:, tt], fc[:, tt], AF.Ln)
                nc.vector.tensor_scalar_mul(fc[:, tt], fc[:, tt], 0.125)

        st_pool = ctx.enter_context(tc.tile_pool(name="state", bufs=1))
        hk = st_pool.tile([128, HP, 2, M], F32)
        hv = st_pool.tile([128, V], F32)
        nc.vector.memset(hk[:], 0.0)
        nc.vector.memset(hv[:], 0.0)

        with tc.tile_pool(name="rw", bufs=3) as rw, \
             tc.tile_pool(name="psA", bufs=3, space="PSUM") as psA, \
             tc.tile_pool(name="psB", bufs=1, space="PSUM") as psB, \
             tc.tile_pool(name="psO", bufs=1, space="PSUM") as psO, \
             tc.tile_pool(name="psS", bufs=3, space="PSUM") as psS:
            for i in range(NCH):
                ci = slice(i * 128, (i + 1) * 128)
                fch = fc[:, i]
                fTch = fT[:, ci]

                ps_cum = psA.tile([128, 128], F32, tag="psA")
                nc.tensor.matmul(ps_cum[:], U[:], fch, start=True, stop=True)
                ps_cumT = psA.tile([128, 128], F32, tag="psA")
                nc.tensor.matmul(ps_cumT[:], fch, U[:], start=True, stop=True)

                Ai = rw.tile([128, 128], F32, tag="Ai")
                nc.scalar.activation(Ai[:], ps_cum[:], AF.Exp)
                AiI = rw.tile([128, 128], F32, tag="AiI")
                nc.scalar.activation(AiI[:], ps_cum[:], AF.Exp, scale=-1.0)
                AiTI = rw.tile([128, 128], F32, tag="AiTI")
                nc.scalar.activation(AiTI[:], ps_cumT[:], AF.Exp, scale=-1.0)
                atotT = rw.tile([128, 1], F32, tag="atotT")
                nc.scalar.activation(atotT[:], ps_cumT[:, 127:128], AF.Exp)

                ef = rw.tile([128, 128], F32, tag="ef")
                nc.scalar.activation(ef[:], fch, AF.Exp)
                efT = rw.tile([128, 128], F32, tag="efT")
                nc.scalar.activation(efT[:], fTch, AF.Exp)
                stil = rw.tile([128, 128], F32, tag="stil")
                nc.vector.tensor_mul(stil[:], ef[:], AiI[:])
                nc.vector.tensor_sub(stil[:], AiI[:], stil[:])
                stilT = rw.tile([128, 128], F32, tag="stilT")
                nc.vector.tensor_mul(stilT[:], efT[:], AiTI[:])
                nc.vector.tensor_sub(stilT[:], AiTI[:], stilT[:])

                ps_b = psB.tile([128, 128], F32, tag="psB")
                nc.tensor.matmul(ps_b[:], ones[:], fch, start=True, stop=True)
                atot_b = rw.tile([128, 128], F32, tag="atot_b")
                nc.scalar.activation(atot_b[:], ps_b[:], AF.Exp)
                sa16 = rw.tile([128, 128], F16, tag="sa16")
                nc.vector.tensor_mul(sa16[:], stil[:], atot_b[:])

                logits = rw.tile([128, HP, M], F32, tag="logits")
                for h in range(HP):
                    msl = slice(h * M, (h + 1) * M)
                    ps_qk = psA.tile([128, 128], F32, tag="psA")
                    nc.tensor.matmul(ps_qk[:], kT[:, 2 * h, ci], qT[:, 2 * h, ci],
                                     start=True, stop=False)
                    nc.tensor.matmul(ps_qk[:], kT[:, 2 * h + 1, ci],
                                     qT[:, 2 * h + 1, ci], start=False, stop=True)
                    QKmT = rw.tile([128, 128], F32, tag=f"QKmT{h}")
                    nc.vector.tensor_mul(QKmT[:], ps_qk[:], U[:])
                    ps_l = psA.tile([128, M], F32, tag="psA")
                    nc.tensor.matmul(ps_l[:], qT[:, 2 * h, ci], hk[:, h, 0],
                                     start=True, stop=False)
                    nc.tensor.matmul(ps_l[:], qT[:, 2 * h + 1, ci], hk[:, h, 1],
                                     start=False, stop=False)
                    nc.tensor.matmul(ps_l[:], QKmT[:], stil[:, msl],
                                     start=False, stop=True)
                    nc.vector.tensor_mul(logits[:, h], ps_l[:], Ai[:, msl])

                nmx = rw.tile([128, HP], F32, tag="nmx")
                nc.vector.tensor_reduce(out=nmx[:], in_=logits[:], axis=AX.X,
                                        op=ALU.max)
                nc.vector.tensor_scalar_mul(nmx[:], nmx[:], -1.0)
                e_t = rw.tile([128, HP, M], F32, tag="e_t")
                den = rw.tile([128, HP], F32, tag="den")
                for h in range(HP):
                    nc.scalar.activation(e_t[:, h], logits[:, h], AF.Exp,
                                         bias=nmx[:, h:h + 1],
                                         accum_out=den[:, h:h + 1])
                rec = rw.tile([128, HP], F32, tag="rec")
                nc.vector.reciprocal(rec[:], den[:])
                pt = rw.tile([128, HP, M], F32, tag="pt")
                nc.vector.tensor_mul(pt[:], e_t[:],
                                     Ai[:].rearrange("p (a b) -> p a b", a=HP))
                for h in range(HP):
                    nc.vector.tensor_scalar_mul(pt[:, h], pt[:, h], rec[:, h:h + 1])

                ps_t = psA.tile([128, 128], F32, tag="psA")
                nc.tensor.transpose(ps_t[:], pt[:].rearrange("p a b -> p (a b)"),
                                    ident[:])
                ptT = rw.tile([128, 128], F32, tag="ptT")
                nc.vector.tensor_copy(ptT[:], ps_t[:])

                for h in range(HP):
                    m2 = slice(h * M, (h + 1) * M)
                    vsl = slice(h * V, (h + 1) * V)
                    ps_ps = psA.tile([128, 128], F32, tag="psA")
                    nc.tensor.matmul(ps_ps[:], stilT[m2, :], ptT[m2, :],
                                     start=True, stop=True)
                    PSmT = rw.tile([128, 128], F16, tag="PSmT")
                    nc.vector.tensor_mul(PSmT[:], ps_ps[:], U[:])
                    ps_o = psO.tile([128, V], F32, tag="psO")
                    nc.tensor.matmul(ps_o[:], ptT[m2, :], hv[m2, :],
                                     start=True, stop=False)
                    nc.tensor.matmul(ps_o[:], PSmT[:], vc[:, i, vsl],
                                     start=False, stop=True)
                    sq = rw.tile([128, V], F32, tag="sq")
                    nc.scalar.activation(sq[:], ps_o[:], AF.Square)
                    ss = rw.tile([128, 1], F32, tag="ss")
                    nc.vector.reduce_sum(out=ss[:], in_=sq[:], axis=AX.X)
                    nc.scalar.activation(ss[:], ss[:], AF.Ln, scale=1.0 / V,
                                         bias=epsc[:])
                    rstd = rw.tile([128, 1], F32, tag="rstd")
                    nc.scalar.activation(rstd[:], ss[:], AF.Exp, scale=-0.5)
                    nc.vector.tensor_scalar_mul(on16[:, i, vsl], ps_o[:], rstd[:])

                ps_hv = psS.tile([128, V], F32, tag="psS")
                for h in range(HP):
                    m2 = slice(h * M, (h + 1) * M)
                    vsl = slice(h * V, (h + 1) * V)
                    for j in range(2):
                        ksl = slice(h * 256 + j * 128, h * 256 + (j + 1) * 128)
                        ps_k = psS.tile([128, M], F32, tag="psS")
                        nc.tensor.matmul(ps_k[:], kc[:, i, ksl], sa16[:, m2],
                                         start=True, stop=True)
                        tmpk = rw.tile([128, M], F32, tag="tmpk")
                        nc.vector.tensor_mul(tmpk[:], hk[:, h, j], atot_b[:, m2])
                        nc.vector.tensor_add(hk[:, h, j], tmpk[:], ps_k[:])
                    nc.tensor.matmul(ps_hv[m2, :], sa16[:, m2], vc[:, i, vsl],
                                     start=True, stop=True)
                tmpv = rw.tile([128, V], F32, tag="tmpv")
                nc.vector.tensor_scalar_mul(tmpv[:], hv[:], atotT[:])
                nc.vector.tensor_add(hv[:], tmpv[:], ps_hv[:])

        onh = dram.tile([T, FQ], F16)
        nc.sync.dma_start(out=onh.rearrange("(a p) f -> p a f", p=128),
                          in_=on16[:])
        with tc.tile_pool(name="onTp", bufs=1) as onTp:
            onT = onTp.tile([128, 4, T], F16)
            for j in range(4):
                nc.sync.dma_start(out=onT[:, j],
                                  in_=onh[:, j * 128:(j + 1) * 128],
                                  transpose=True)
            onTh = dram.tile([FQ, T], F16)
            nc.sync.dma_start(out=onTh.rearrange("(j p) t -> p j t", p=128),
                              in_=onT[:])
            onTfull = dram.tile([2 * FQ, T], F16)
            nc.gpsimd.collective_compute(
                "AllGather", ALU.bypass, replica_groups=PAIRS,
                ins=[onTh.opt()], outs=[onTfull.opt()])

            io2 = io.rearrange("a (r b) -> (a r) b", r=2)
            with tc.tile_pool(name="fml", bufs=10) as fml, \
                 tc.tile_pool(name="fwo", bufs=1) as fwo, \
                 tc.tile_pool(name="fob", bufs=3) as fob, \
                 tc.tile_pool(name="fps", bufs=2, space="PSUM") as fps:
                woc = fwo.tile([128, 8, 512], F16)
                for hvt in range(8):
                    nc.sync.dma_start(
                        out=woc[:, hvt],
                        in_=wf[hvt * 128:(hvt + 1) * 128, OO:OO + 512])
                for tt in range(T // 128):
                    lts = []
                    for hvt in range(8):
                        lt = fml.tile([128, 128], F16, tag="fml")
                        nc.sync.dma_start(
                            out=lt[:], in_=onTfull[hvt * 128:(hvt + 1) * 128,
                                                   tt * 128:(tt + 1) * 128])
                        lts.append(lt)
                    ps = fps.tile([128, 512], F32, tag="fps")
                    for hvt in range(8):
                        nc.tensor.matmul(ps[:], lts[hvt][:], woc[:, hvt],
                                         start=(hvt == 0), stop=(hvt == 7))
                    ot = fob.tile([128, 512], F16, tag="fob")
                    nc.scalar.activation(ot[:], ps[:], AF.Copy)
                    nc.sync.dma_start(out=io2[tt * 128:(tt + 1) * 128, :],
                                      in_=ot[:])
    nc.compile()
    return nc


def _make_runner(nc):
    import jax
    import concourse.mybir as mybir
    from concourse.bass2jax import (_bass_exec_p, install_neuronx_cc_hook,
                                    partition_id_tensor)
    from jax.sharding import Mesh, PartitionSpec
    from jax.experimental.shard_map import shard_map

    install_neuronx_cc_hook()
    partition_name = nc.partition_id_tensor.name if nc.partition_id_tensor else None
    in_names, out_names, out_avals = [], [], []
    for alloc in nc.m.functions[0].allocations:
        if not isinstance(alloc, mybir.MemoryLocationSet):
            continue
        name = alloc.memorylocations[0].name
        if alloc.kind == "ExternalInput":
            if name != partition_name:
                in_names.append(name)
        elif alloc.kind == "ExternalOutput":
            out_names.append(name)
            out_avals.append(jax.core.ShapedArray(tuple(alloc.tensor_shape),
                                                  mybir.dt.np(alloc.dtype)))
    n_params = len(in_names)
    n_outs = len(out_names)
    all_in_names = tuple(in_names + out_names +
                         ([partition_name] if partition_name else []))

    def _body(*args):
        operands = list(args)
        if partition_name:
            operands.append(partition_id_tensor())
        return tuple(_bass_exec_p.bind(
            *operands, out_avals=tuple(out_avals), in_names=all_in_names,
            out_names=tuple(out_names), lowering_input_output_aliases=(),
            sim_require_finite=True, sim_require_nnan=True, nc=nc))

    devices = jax.devices()[:8]
    mesh = Mesh(np.asarray(devices), ("core",))
    donate = tuple(range(n_params, n_params + n_outs))
    sharded = jax.jit(
        shard_map(_body, mesh=mesh,
                  in_specs=(PartitionSpec("core"),) * (n_params + n_outs),
                  out_specs=(PartitionSpec("core"),) * n_outs, check_rep=False),
        donate_argnums=donate, keep_unused=True)

    def run(wblob_g, io_g):
        out_arrs = sharded(wblob_g, io_g)
        return np.asarray(out_arrs[0])          # [8*TH, D] f16

    return run


_RUN = None
try:
    _NC = _build()
    _RUN = _make_runner(_NC)
    # warm: compile-cache load, axon transfer channels, jit cache
    _RUN(np.zeros((8 * WROWS, WCOLS), np.float16),
         np.zeros((8 * TH, D), np.float16))
except Exception:
    _RUN = None


def _device_kernel(x, Wq, Wk, Wv, Wf, g_norm_w, Wo):
    xf = np.asarray(x, np.float32).astype(np.float16)       # [B, T, D]
    Wo2 = (np.asarray(Wo, np.float32) *
           np.tile(np.asarray(g_norm_w, np.float32), H)[:, None])
    blobs = []
    for hp in range(2):
        qs = slice(hp * FQ, (hp + 1) * FQ)
        ms = slice(hp * FM, (hp + 1) * FM)
        blob = np.concatenate([
            np.asarray(Wq, np.float32)[:, qs], np.asarray(Wk, np.float32)[:, qs],
            np.asarray(Wv, np.float32)[:, qs], np.asarray(Wf, np.float32)[:, ms],
            Wo2[:, hp * 512:(hp + 1) * 512]], axis=1).astype(np.float16)
        blobs.append(blob)
    wblob_g = np.empty((8 * WROWS, WCOLS), np.float16)
    io_g = np.empty((8 * TH, D), np.float16)
    for c in range(8):
        b, hp = c // 2, c % 2
        wblob_g[c * WROWS:(c + 1) * WROWS] = \
            blobs[hp][(c // 2) * WROWS:(c // 2 + 1) * WROWS]
        io_g[c * TH:(c + 1) * TH] = xf[b, hp * TH:(hp + 1) * TH]
    res = _RUN(wblob_g, io_g)                               # [8*TH, D] f16
    out = np.empty((B, T, D), np.float32)
    for c in range(8):
        b, hp = c // 2, c % 2
        out[b, :, hp * 512:(hp + 1) * 512] = \
            res[c * TH:(c + 1) * TH].reshape(T, 512).astype(np.float32)
    return out


# ---------------- numpy fallback (chunkwise, C=64) ----------------
def _numpy_kernel(x, Wq, Wk, Wv, Wf, g_norm_w, Wo):
    x = np.asarray(x, np.float32)
    Wq, Wk, Wv, Wf = (np.asarray(w, np.float32) for w in (Wq, Wk, Wv, Wf))
    gw, Wo = np.asarray(g_norm_w, np.float32), np.asarray(Wo, np.float32)
    Cn = 64
    ncn = T // Cn
    sig = lambda z: 1.0 / (1.0 + np.exp(-z))
    out = np.empty((B, T, D), np.float32)
    mask = np.tril(np.ones((Cn, Cn), np.float32))
    for b in range(B):
        xb = x[b]
        q = (lambda y: y * sig(y))(xb @ Wq).reshape(T, H, K) * SCALE
        k = (lambda y: y * sig(y))(xb @ Wk).reshape(T, H, K)
        v = (xb @ Wv).reshape(T, H, V)
        f = (-np.logaddexp(0.0, -(xb @ Wf)) / 8.0).reshape(T, H, M)
        s = 1.0 - np.exp(f)
        qc, kc, vc, sc, fc = (a.reshape(ncn, Cn, H, -1).transpose(0, 2, 1, 3)
                              for a in (q, k, v, s, f))
        hk = np.zeros((H, K, M), np.float32)
        hv = np.zeros((H, M, V), np.float32)
        on = np.zeros((ncn, H, Cn, V), np.float32)
        for i in range(ncn):
            Ai = np.exp(np.cumsum(fc[i], axis=1))
            atot = Ai[:, -1, :]
            stil = sc[i] / Ai
            QK = np.einsum('hck,hdk->hcd', qc[i], kc[i]) * mask
            logits = Ai * (np.einsum('hck,hkm->hcm', qc[i], hk)
                           + np.einsum('hcd,hdm->hcm', QK, stil))
            logits -= logits.max(-1, keepdims=True)
            p = np.exp(logits)
            p /= p.sum(-1, keepdims=True)
            ptl = p * Ai
            PS = np.einsum('hcm,hdm->hcd', ptl, stil) * mask
            o = (np.einsum('hcm,hmv->hcv', ptl, hv)
                 + np.einsum('hcd,hdv->hcv', PS, vc[i]))
            sa = stil * atot[:, None, :]
            hk = hk * atot[:, None, :] + np.einsum('hck,hcm->hkm', kc[i], sa)
            hv = hv * atot[:, :, None] + np.einsum('hcm,hcv->hmv', sa, vc[i])
            on[i] = o
        o = on.transpose(0, 2, 1, 3).reshape(T, H, V)
        o = o / np.sqrt((o * o).mean(-1, keepdims=True) + EPS) * gw
        out[b] = o.reshape(T, H * V) @ Wo
    return out


def kernel(x, Wq, Wk, Wv, Wf, g_norm_w, Wo):
    if _RUN is not None:
        try:
            return _device_kernel(x, Wq, Wk, Wv, Wf, g_norm_w, Wo)
        except Exception:
            pass
    return _numpy_kernel(x, Wq, Wk, Wv, Wf, g_norm_w, Wo)


# revision 3
# speedup vs baseline: 2.5628x; 1.0262x over previous
"""GSA block kernel for 8 trn2 NeuronCores (Bass/Tile via PJRT).

Sharding: core c -> batch b=c//2, head-pair hp=c%2 (heads {2hp, 2hp+1}).
Wire fp16: per-core "io" (ExternalOutput donated with x half-batch rows as
its initial value; final out D-column half written back) + "wblob" (1/4
slice of the per-hp weight blob). On-device: pair AllGather rebuilds x[b];
quad AllGather ({0,2,4,6}/{1,3,5,7}) rebuilds the hp weight blob; chunkwise
(C=128) recurrence in fp32; pair AllGather of o_n^T; each core computes its
512 output columns (Wo half rides in the hp blob, so the program is
core-independent). Falls back to a pure-numpy path if the device path is
unavailable.
"""
import contextlib
import numpy as np

B, T, D = 4, 2048, 1024
H, K, V, M = 4, 256, 256, 64
HP = 2
FQ = HP * K              # 512
FM = HP * M              # 128
CC = 128                 # device chunk length
NCH = T // CC            # 16
SCALE = K ** -0.5
EPS = 1e-5
TH = T // 2              # 1024

OQ, OKk, OV, OF, OO = 0, 512, 1024, 1536, 1664
WCOLS = 2176
WROWS = 256

PAIRS = [[0, 1], [2, 3], [4, 5], [6, 7]]
QUADS = [[0, 2, 4, 6], [1, 3, 5, 7]]


def _build():
    import concourse.mybir as mybir
    import concourse.tile as tile
    from concourse import bacc
    from concourse.masks import make_identity, make_upper_triangular

    F16 = mybir.dt.float16
    F32 = mybir.dt.float32
    AF = mybir.ActivationFunctionType
    ALU = mybir.AluOpType
    AX = mybir.AxisListType

    nc = bacc.Bacc("TRN2", target_bir_lowering=False, debug=False, num_devices=8)
    wblob = nc.dram_tensor("wblob", [WROWS, WCOLS], F16, kind="ExternalInput").ap()
    io = nc.dram_tensor("io", [TH, D], F16, kind="ExternalOutput").ap()

    with tile.TileContext(nc) as tc, contextlib.ExitStack() as ctx:
        dram = ctx.enter_context(tc.tile_pool(name="dram", bufs=1, space="DRAM"))
        consts = ctx.enter_context(tc.tile_pool(name="consts", bufs=1))

        xh = dram.tile([TH, D], F16)
        nc.sync.dma_start(out=xh[:], in_=io[:])
        xb = dram.tile([T, D], F16)
        nc.gpsimd.collective_compute(
            "AllGather", ALU.bypass, replica_groups=PAIRS,
            ins=[xh.opt()], outs=[xb.opt()])
        wh = dram.tile([WROWS, WCOLS], F16)
        nc.sync.dma_start(out=wh[:], in_=wblob[:])
        wf = dram.tile([4 * WROWS, WCOLS], F16)
        nc.gpsimd.collective_compute(
            "AllGather", ALU.bypass, replica_groups=QUADS,
            ins=[wh.opt()], outs=[wf.opt()])

        U = consts.tile([128, 128], F32)
        make_upper_triangular(nc, U[:], val=1.0, diag=True)
        ident = consts.tile([128, 128], F32)
        make_identity(nc, ident[:])
        ones = consts.tile([128, 128], F32)
        nc.vector.memset(ones[:], 1.0)
        epsc = consts.tile([128, 1], F32)
        nc.vector.memset(epsc[:], EPS)

        xT_pool = ctx.enter_context(tc.tile_pool(name="xTp", bufs=1))
        xT = xT_pool.tile([128, 8, T], F16)
        for d8 in range(8):
            nc.sync.dma_start(out=xT[:, d8], in_=xb[:, d8 * 128:(d8 + 1) * 128],
                              transpose=True)

        res_pool = ctx.enter_context(tc.tile_pool(name="resp", bufs=1))
        qT = res_pool.tile([128, 4, T], F32)
        kT = res_pool.tile([128, 4, T], F32)
        fT = res_pool.tile([128, T], F32)
        kc = res_pool.tile([128, NCH, FQ], F16)
        vc = res_pool.tile([128, NCH, FQ], F16)
        fc = res_pool.tile([128, NCH, FM], F32)
        on16 = res_pool.tile([128, NCH, FQ], F16)

        with tc.tile_pool(name="pjw", bufs=10) as pjw, \
             tc.tile_pool(name="pjwa", bufs=1) as pjwa, \
             tc.tile_pool(name="pjps", bufs=4, space="PSUM") as pjps:

            def projB(colbase, nf, evict):
                for ft in range(nf):
                    wts = []
                    for d8 in range(8):
                        w_ = pjw.tile([128, 128], F16, tag="pjw")
                        nc.sync.dma_start(
                            out=w_[:],
                            in_=wf[d8 * 128:(d8 + 1) * 128,
                                   colbase + ft * 128:colbase + (ft + 1) * 128])
                        wts.append(w_)
                    for tk in range(4):
                        ps = pjps.tile([128, 512], F32, tag="pjps")
                        for d8 in range(8):
                            nc.tensor.matmul(ps[:], wts[d8][:],
                                             xT[:, d8, tk * 512:(tk + 1) * 512],
                                             start=(d8 == 0), stop=(d8 == 7))
                        evict(ps, ft, tk)

            def ev_q(ps, ft, tk):
                sl = qT[:, ft, tk * 512:(tk + 1) * 512]
                nc.scalar.activation(sl, ps[:], AF.Silu)
                nc.vector.tensor_scalar_mul(sl, sl, SCALE)

            def ev_k(ps, ft, tk):
                nc.scalar.activation(kT[:, ft, tk * 512:(tk + 1) * 512], ps[:],
                                     AF.Silu)

            def ev_fT(ps, ft, tk):
                sl = fT[:, tk * 512:(tk + 1) * 512]
                nc.scalar.activation(sl, ps[:], AF.Sigmoid)
                nc.scalar.activation(sl, sl, AF.Ln)
                nc.vector.tensor_scalar_mul(sl, sl, 0.125)

            projB(OQ, 4, ev_q)
            projB(OKk, 4, ev_k)
            projB(OF, 1, ev_fT)

            wkA = pjwa.tile([128, 8, FQ], F16)
            wvA = pjwa.tile([128, 8, FQ], F16)
            wfA = pjwa.tile([128, 8, FM], F16)
            for d8 in range(8):
                r = slice(d8 * 128, (d8 + 1) * 128)
                nc.sync.dma_start(out=wkA[:, d8], in_=wf[r, OKk:OKk + FQ])
                nc.sync.dma_start(out=wvA[:, d8], in_=wf[r, OV:OV + FQ])
                nc.sync.dma_start(out=wfA[:, d8], in_=wf[r, OF:OF + FM])
            for tt in range(NCH):
                tsl = slice(tt * 128, (tt + 1) * 128)
                psk = pjps.tile([128, FQ], F32, tag="pjps")
                for d8 in range(8):
                    nc.tensor.matmul(psk[:], xT[:, d8, tsl], wkA[:, d8],
                                     start=(d8 == 0), stop=(d8 == 7))
                nc.scalar.activation(kc[:, tt], psk[:], AF.Silu)
                psv = pjps.tile([128, FQ], F32, tag="pjps")
                for d8 in range(8):
                    nc.tensor.matmul(psv[:], xT[:, d8, tsl], wvA[:, d8],
                                     start=(d8 == 0), stop=(d8 == 7))
                nc.scalar.activation(vc[:, tt], psv[:], AF.Copy)
                psf = pjps.tile([128, FM], F32, tag="pjps")
                for d8 in range(8):
                    nc.tensor.matmul(psf[:], xT[:, d8, tsl], wfA[:, d8],
                                     start=(d8 == 0), stop=(d8 == 7))
                nc.scalar.activation(fc[:, tt], psf[:], AF.Sigmoid)
                nc.scalar.activation(fc[:, tt], fc[:, tt], AF.Ln)
                nc.vector.tensor_scalar_mul(fc[:, tt], fc[:, tt], 0.125)

        st_pool = ctx.enter_context(tc.tile_pool(name="state", bufs=1))
        hk = st_pool.tile([128, HP, 2, M], F32)
        hv = st_pool.tile([128, V], F32)
        nc.vector.memset(hk[:], 0.0)
        nc.vector.memset(hv[:], 0.0)

        with tc.tile_pool(name="rw", bufs=3) as rw, \
             tc.tile_pool(name="psA", bufs=3, space="PSUM") as psA, \
             tc.tile_pool(name="psB", bufs=1, space="PSUM") as psB, \
             tc.tile_pool(name="psO", bufs=1, space="PSUM") as psO, \
             tc.tile_pool(name="psS", bufs=3, space="PSUM") as psS:
            for i in range(NCH):
                ci = slice(i * 128, (i + 1) * 128)
                fch = fc[:, i]
                fTch = fT[:, ci]

                ps_cum = psA.tile([128, 128], F32, tag="psA")
                nc.tensor.matmul(ps_cum[:], U[:], fch, start=True, stop=True)
                ps_cumT = psA.tile([128, 128], F32, tag="psA")
                nc.tensor.matmul(ps_cumT[:], fch, U[:], start=True, stop=True)

                Ai = rw.tile([128, 128], F32, tag="Ai")
                nc.scalar.activation(Ai[:], ps_cum[:], AF.Exp)
                AiI = rw.tile([128, 128], F32, tag="AiI")
                nc.scalar.activation(AiI[:], ps_cum[:], AF.Exp, scale=-1.0)
                AiTI = rw.tile([128, 128], F32, tag="AiTI")
                nc.scalar.activation(AiTI[:], ps_cumT[:], AF.Exp, scale=-1.0)
                atotT = rw.tile([128, 1], F32, tag="atotT")
                nc.scalar.activation(atotT[:], ps_cumT[:, 127:128], AF.Exp)

                ef = rw.tile([128, 128], F32, tag="ef")
                nc.scalar.activation(ef[:], fch, AF.Exp)
                efT = rw.tile([128, 128], F32, tag="efT")
                nc.scalar.activation(efT[:], fTch, AF.Exp)
                stil = rw.tile([128, 128], F32, tag="stil")
                nc.vector.tensor_mul(stil[:], ef[:], AiI[:])
                nc.vector.tensor_sub(stil[:], AiI[:], stil[:])
                stilT = rw.tile([128, 128], F32, tag="stilT")
                nc.vector.tensor_mul(stilT[:], efT[:], AiTI[:])
                nc.vector.tensor_sub(stilT[:], AiTI[:], stilT[:])

                ps_b = psB.tile([128, 128], F32, tag="psB")
                nc.tensor.matmul(ps_b[:], ones[:], fch, start=True, stop=True)
                atot_b = rw.tile([128, 128], F32, tag="atot_b")
                nc.scalar.activation(atot_b[:], ps_b[:], AF.Exp)
                sa16 = rw.tile([128, 128], F16, tag="sa16")
                nc.vector.tensor_mul(sa16[:], stil[:], atot_b[:])

                logits = rw.tile([128, HP, M], F32, tag="logits")
                for h in range(HP):
                    msl = slice(h * M, (h + 1) * M)
                    ps_qk = psA.tile([128, 128], F32, tag="psA")
                    nc.tensor.matmul(ps_qk[:], kT[:, 2 * h, ci], qT[:, 2 * h, ci],
                                     start=True, stop=False)
                    nc.tensor.matmul(ps_qk[:], kT[:, 2 * h + 1, ci],
                                     qT[:, 2 * h + 1, ci], start=False, stop=True)
                    QKmT = rw.tile([128, 128], F32, tag=f"QKmT{h}")
                    nc.vector.tensor_mul(QKmT[:], ps_qk[:], U[:])
                    ps_l = psA.tile([128, M], F32, tag="psA")
                    nc.tensor.matmul(ps_l[:], qT[:, 2 * h, ci], hk[:, h, 0],
                                     start=True, stop=False)
                    nc.tensor.matmul(ps_l[:], qT[:, 2 * h + 1, ci], hk[:, h, 1],
                                     start=False, stop=False)
                    nc.tensor.matmul(ps_l[:], QKmT[:], stil[:, msl],
                                     start=False, stop=True)
                    nc.vector.tensor_mul(logits[:, h], ps_l[:], Ai[:, msl])

                nmx = rw.tile([128, HP], F32, tag="nmx")
                nc.vector.tensor_reduce(out=nmx[:], in_=logits[:], axis=AX.X,
                                        op=ALU.max)
                nc.vector.tensor_scalar_mul(nmx[:], nmx[:], -1.0)
                e_t = rw.tile([128, HP, M], F32, tag="e_t")
                den = rw.tile([128, HP], F32, tag="den")
                for h in range(HP):
                    nc.scalar.activation(e_t[:, h], logits[:, h], AF.Exp,
                                         bias=nmx[:, h:h + 1],
                                         accum_out=den[:, h:h + 1])
                rec = rw.tile([128, HP], F32, tag="rec")
                nc.vector.reciprocal(rec[:], den[:])
                pt = rw.tile([128, HP, M], F32, tag="pt")
                nc.vector.tensor_mul(pt[:], e_t[:],
                                     Ai[:].rearrange("p (a b) -> p a b", a=HP))
                for h in range(HP):
                    nc.vector.tensor_scalar_mul(pt[:, h], pt[:, h], rec[:, h:h + 1])

                ps_t = psA.tile([128, 128], F32, tag="psA")
                nc.tensor.transpose(ps_t[:], pt[:].rearrange("p a b -> p (a b)"),
                                    ident[:])
                ptT = rw.tile([128, 128], F32, tag="ptT")
                nc.vector.tensor_copy(ptT[:], ps_t[:])

                for h in range(HP):
                    m2 = slice(h * M, (h + 1) * M)
                    vsl = slice(h * V, (h + 1) * V)
                    ps_ps = psA.tile([128, 128], F32, tag="psA")
                    nc.tensor.matmul(ps_ps[:], stilT[m2, :], ptT[m2, :],
                                     start=True, stop=True)
                    PSmT = rw.tile([128, 128], F16, tag="PSmT")
                    nc.vector.tensor_mul(PSmT[:], ps_ps[:], U[:])
                    ps_o = psO.tile([128, V], F32, tag="psO")
                    nc.tensor.matmul(ps_o[:], ptT[m2, :], hv[m2, :],
                                     start=True, stop=False)
                    nc.tensor.matmul(ps_o[:], PSmT[:], vc[:, i, vsl],
                                     start=False, stop=True)
                    sq = rw.tile([128, V], F32, tag="sq")
                    nc.scalar.activation(sq[:], ps_o[:], AF.Square)
                    ss = rw.tile([128, 1], F32, tag="ss")
                    nc.vector.reduce_sum(out=ss[:], in_=sq[:], axis=AX.X)
                    nc.scalar.activation(ss[:], ss[:], AF.Ln, scale=1.0 / V,
                                         bias=epsc[:])
                    rstd = rw.tile([128, 1], F32, tag="rstd")
                    nc.scalar.activation(rstd[:], ss[:], AF.Exp, scale=-0.5)
                    nc.vector.tensor_scalar_mul(on16[:, i, vsl], ps_o[:], rstd[:])

                ps_hv = psS.tile([128, V], F32, tag="psS")
                for h in range(HP):
                    m2 = slice(h * M, (h + 1) * M)
                    vsl = slice(h * V, (h + 1) * V)
                    for j in range(2):
                        ksl = slice(h * 256 + j * 128, h * 256 + (j + 1) * 128)
                        ps_k = psS.tile([128, M], F32, tag="psS")
                        nc.tensor.matmul(ps_k[:], kc[:, i, ksl], sa16[:, m2],
                                         start=True, stop=True)
                        tmpk = rw.tile([128, M], F32, tag="tmpk")
                        nc.vector.tensor_mul(tmpk[:], hk[:, h, j], atot_b[:, m2])
                        nc.vector.tensor_add(hk[:, h, j], tmpk[:], ps_k[:])
                    nc.tensor.matmul(ps_hv[m2, :], sa16[:, m2], vc[:, i, vsl],
                                     start=True, stop=True)
                tmpv = rw.tile([128, V], F32, tag="tmpv")
                nc.vector.tensor_scalar_mul(tmpv[:], hv[:], atotT[:])
                nc.vector.tensor_add(hv[:], tmpv[:], ps_hv[:])

        onh = dram.tile([T, FQ], F16)
        nc.sync.dma_start(out=onh.rearrange("(a p) f -> p a f", p=128),
                          in_=on16[:])
        with tc.tile_pool(name="onTp", bufs=1) as onTp:
            onT = onTp.tile([128, 4, T], F16)
            for j in range(4):
                nc.sync.dma_start(out=onT[:, j],
                                  in_=onh[:, j * 128:(j + 1) * 128],
                                  transpose=True)
            onTh = dram.tile([FQ, T], F16)
            nc.sync.dma_start(out=onTh.rearrange("(j p) t -> p j t", p=128),
                              in_=onT[:])
            onTfull = dram.tile([2 * FQ, T], F16)
            nc.gpsimd.collective_compute(
                "AllGather", ALU.bypass, replica_groups=PAIRS,
                ins=[onTh.opt()], outs=[onTfull.opt()])

            io2 = io.rearrange("a (r b) -> (a r) b", r=2)
            with tc.tile_pool(name="fml", bufs=10) as fml, \
                 tc.tile_pool(name="fwo", bufs=1) as fwo, \
                 tc.tile_pool(name="fob", bufs=3) as fob, \
                 tc.tile_pool(name="fps", bufs=2, space="PSUM") as fps:
                woc = fwo.tile([128, 8, 512], F16)
                for hvt in range(8):
                    nc.sync.dma_start(
                        out=woc[:, hvt],
                        in_=wf[hvt * 128:(hvt + 1) * 128, OO:OO + 512])
                for tt in range(T // 128):
                    lts = []
                    for hvt in range(8):
                        lt = fml.tile([128, 128], F16, tag="fml")
                        nc.sync.dma_start(
                            out=lt[:], in_=onTfull[hvt * 128:(hvt + 1) * 128,
                                                   tt * 128:(tt + 1) * 128])
                        lts.append(lt)
                    ps = fps.tile([128, 512], F32, tag="fps")
                    for hvt in range(8):
                        nc.tensor.matmul(ps[:], lts[hvt][:], woc[:, hvt],
                                         start=(hvt == 0), stop=(hvt == 7))
                    ot = fob.tile([128, 512], F16, tag="fob")
                    nc.scalar.activation(ot[:], ps[:], AF.Copy)
                    nc.sync.dma_start(out=io2[tt * 128:(tt + 1) * 128, :],
                                      in_=ot[:])
    nc.compile()
    return nc


def _make_runner(nc):
    import jax
    import concourse.mybir as mybir
    from concourse.bass2jax import (_bass_exec_p, install_neuronx_cc_hook,
                                    partition_id_tensor)
    from jax.sharding import Mesh, PartitionSpec
    from jax.experimental.shard_map import shard_map

    install_neuronx_cc_hook()
    partition_name = nc.partition_id_tensor.name if nc.partition_id_tensor else None
    in_names, out_names, out_avals = [], [], []
    for alloc in nc.m.functions[0].allocations:
        if not isinstance(alloc, mybir.MemoryLocationSet):
            continue
        name = alloc.memorylocations[0].name
        if alloc.kind == "ExternalInput":
            if name != partition_name:
                in_names.append(name)
        elif alloc.kind == "ExternalOutput":
            out_names.append(name)
            out_avals.append(jax.core.ShapedArray(tuple(alloc.tensor_shape),
                                                  mybir.dt.np(alloc.dtype)))
    n_params = len(in_names)
    n_outs = len(out_names)
    all_in_names = tuple(in_names + out_names +
                         ([partition_name] if partition_name else []))

    def _body(*args):
        operands = list(args)
        if partition_name:
            operands.append(partition_id_tensor())
        return tuple(_bass_exec_p.bind(
            *operands, out_avals=tuple(out_avals), in_names=all_in_names,
            out_names=tuple(out_names), lowering_input_output_aliases=(),
            sim_require_finite=True, sim_require_nnan=True, nc=nc))

    devices = jax.devices()[:8]
    mesh = Mesh(np.asarray(devices), ("core",))
    donate = tuple(range(n_params, n_params + n_outs))
    sharded = jax.jit(
        shard_map(_body, mesh=mesh,
                  in_specs=(PartitionSpec("core"),) * (n_params + n_outs),
                  out_specs=(PartitionSpec("core"),) * n_outs, check_rep=False),
        donate_argnums=donate, keep_unused=True)

    from concurrent.futures import ThreadPoolExecutor
    from jax.sharding import NamedSharding
    sh = NamedSharding(mesh, PartitionSpec("core"))
    pool = ThreadPoolExecutor(16)

    def run(wblob_g, io_g):
        # threaded per-device upload, then assemble global arrays
        def up(args):
            arr, dev = args
            d = jax.device_put(arr, dev)
            d.block_until_ready()
            return d
        jobs = ([(wblob_g[c * WROWS:(c + 1) * WROWS], devices[c]) for c in range(8)]
                + [(io_g[c * TH:(c + 1) * TH], devices[c]) for c in range(8)])
        parts = list(pool.map(up, jobs))
        wglob = jax.make_array_from_single_device_arrays(
            (8 * WROWS, WCOLS), sh, parts[:8])
        ioglob = jax.make_array_from_single_device_arrays(
            (8 * TH, D), sh, parts[8:])
        out = sharded(wglob, ioglob)[0]
        shards = sorted(out.addressable_shards,
                        key=lambda s: s.index[0].start or 0)
        datas = list(pool.map(lambda s: np.asarray(s.data), shards))
        return np.concatenate(datas, axis=0)    # [8*TH, D] f16

    return run


_RUN = None
try:
    _NC = _build()
    _RUN = _make_runner(_NC)
    # warm: compile-cache load, axon transfer channels, jit cache
    _RUN(np.zeros((8 * WROWS, WCOLS), np.float16),
         np.zeros((8 * TH, D), np.float16))
except Exception:
    _RUN = None


def _device_kernel(x, Wq, Wk, Wv, Wf, g_norm_w, Wo):
    xf = np.asarray(x, np.float32).astype(np.float16)       # [B, T, D]
    Wo2 = (np.asarray(Wo, np.float32) *
           np.tile(np.asarray(g_norm_w, np.float32), H)[:, None])
    blobs = []
    for hp in range(2):
        qs = slice(hp * FQ, (hp + 1) * FQ)
        ms = slice(hp * FM, (hp + 1) * FM)
        blob = np.concatenate([
            np.asarray(Wq, np.float32)[:, qs], np.asarray(Wk, np.float32)[:, qs],
            np.asarray(Wv, np.float32)[:, qs], np.asarray(Wf, np.float32)[:, ms],
            Wo2[:, hp * 512:(hp + 1) * 512]], axis=1).astype(np.float16)
        blobs.append(blob)
    wblob_g = np.empty((8 * WROWS, WCOLS), np.float16)
    io_g = np.empty((8 * TH, D), np.float16)
    for c in range(8):
        b, hp = c // 2, c % 2
        wblob_g[c * WROWS:(c + 1) * WROWS] = \
            blobs[hp][(c // 2) * WROWS:(c // 2 + 1) * WROWS]
        io_g[c * TH:(c + 1) * TH] = xf[b, hp * TH:(hp + 1) * TH]
    res = _RUN(wblob_g, io_g)                               # [8*TH, D] f16
    out = np.empty((B, T, D), np.float32)
    for c in range(8):
        b, hp = c // 2, c % 2
        out[b, :, hp * 512:(hp + 1) * 512] = \
            res[c * TH:(c + 1) * TH].reshape(T, 512).astype(np.float32)
    return out


# ---------------- numpy fallback (chunkwise, C=64) ----------------
def _numpy_kernel(x, Wq, Wk, Wv, Wf, g_norm_w, Wo):
    x = np.asarray(x, np.float32)
    Wq, Wk, Wv, Wf = (np.asarray(w, np.float32) for w in (Wq, Wk, Wv, Wf))
    gw, Wo = np.asarray(g_norm_w, np.float32), np.asarray(Wo, np.float32)
    Cn = 64
    ncn = T // Cn
    sig = lambda z: 1.0 / (1.0 + np.exp(-z))
    out = np.empty((B, T, D), np.float32)
    mask = np.tril(np.ones((Cn, Cn), np.float32))
    for b in range(B):
        xb = x[b]
        q = (lambda y: y * sig(y))(xb @ Wq).reshape(T, H, K) * SCALE
        k = (lambda y: y * sig(y))(xb @ Wk).reshape(T, H, K)
        v = (xb @ Wv).reshape(T, H, V)
        f = (-np.logaddexp(0.0, -(xb @ Wf)) / 8.0).reshape(T, H, M)
        s = 1.0 - np.exp(f)
        qc, kc, vc, sc, fc = (a.reshape(ncn, Cn, H, -1).transpose(0, 2, 1, 3)
                              for a in (q, k, v, s, f))
        hk = np.zeros((H, K, M), np.float32)
        hv = np.zeros((H, M, V), np.float32)
        on = np.zeros((ncn, H, Cn, V), np.float32)
        for i in range(ncn):
            Ai = np.exp(np.cumsum(fc[i], axis=1))
            atot = Ai[:, -1, :]
            stil = sc[i] / Ai
            QK = np.einsum('hck,hdk->hcd', qc[i], kc[i]) * mask
            logits = Ai * (np.einsum('hck,hkm->hcm', qc[i], hk)
                           + np.einsum('hcd,hdm->hcm', QK, stil))
            logits -= logits.max(-1, keepdims=True)
            p = np.exp(logits)
            p /= p.sum(-1, keepdims=True)
            ptl = p * Ai
            PS = np.einsum('hcm,hdm->hcd', ptl, stil) * mask
            o = (np.einsum('hcm,hmv->hcv', ptl, hv)
                 + np.einsum('hcd,hdv->hcv', PS, vc[i]))
            sa = stil * atot[:, None, :]
            hk = hk * atot[:, None, :] + np.einsum('hck,hcm->hkm', kc[i], sa)
            hv = hv * atot[:, :, None] + np.einsum('hcm,hcv->hmv', sa, vc[i])
            on[i] = o
        o = on.transpose(0, 2, 1, 3).reshape(T, H, V)
        o = o / np.sqrt((o * o).mean(-1, keepdims=True) + EPS) * gw
        out[b] = o.reshape(T, H * V) @ Wo
    return out


def kernel(x, Wq, Wk, Wv, Wf, g_norm_w, Wo):
    if _RUN is not None:
        try:
            return _device_kernel(x, Wq, Wk, Wv, Wf, g_norm_w, Wo)
        except Exception:
            pass
    return _numpy_kernel(x, Wq, Wk, Wv, Wf, g_norm_w, Wo)
